# revision 1
# baseline (speedup 1.0000x reference)
"""Trainium2 Bass kernel for nn_MultiHeadAttention (B=2, S=2048, D=1024, H=16).

Sharding: 8 cores = 2 batch groups x 4 head groups (4 heads / core).
Host pre-transposes X and the weight slices so the device kernel needs no
on-chip transposes:
  - activations flow feature-major (Q^T, K^T [256, 2048]) through scores,
  - scores are computed transposed (S^T[kk, q]) so softmax-sum comes from a
    ones-column augmented into V via the PV matmul itself,
  - the output projection flips back to seq-major [2048, 1024] partials,
  - host sums the 4 head-group partials per batch and adds the output bias.
All matmuls run in float32r (full PE rate); accumulation stays fp32 in PSUM.
"""

import os
import sys

for _p in ("/opt/trn_rl_repo",):
    if os.path.isdir(_p) and _p not in sys.path:
        sys.path.append(_p)

import numpy as np

import concourse.bacc as bacc
import concourse.mybir as mybir
from concourse.bass_utils import run_bass_kernel_spmd
from concourse.tile import TileContext

F32 = mybir.dt.float32
F32R = mybir.dt.float32r

B, S, D, H, HD = 2, 2048, 1024, 16, 64
NCORES = 8
BG = 2                 # batch groups
HG = NCORES // BG      # head groups (cores per batch)
HL = H // HG           # heads per core = 4
DL = HL * HD           # local head dims = 256
KC = D // 128          # contraction chunks for the projections = 8
QB = 512               # q block (matmul free dim)
NQB = S // QB          # 4
KT = 128               # key tile (scores partition dim)
NKT = S // KT          # 16
QT = 128               # out-proj q tile
NQT = S // QT          # 16
SCALE = 1.0 / np.sqrt(HD)

_prog_cache = {}


def _build(is_causal, with_bqk, with_bv, repeat=1):
    nc = bacc.Bacc(None, target_bir_lowering=False, debug=False)

    xq_d = nc.declare_dram_parameter("xqT", [128, NQB * KC * QB], F32, isOutput=False)
    xk_d = nc.declare_dram_parameter("xkT", [128, NQB * KC * QB], F32, isOutput=False)
    xv_d = nc.declare_dram_parameter("xvT", [128, NQT * KC * 128], F32, isOutput=False)
    wq_d = nc.declare_dram_parameter("wqT", [128, KC * DL], F32, isOutput=False)
    wk_d = nc.declare_dram_parameter("wkT", [128, KC * DL], F32, isOutput=False)
    wv_d = nc.declare_dram_parameter("wvT", [128, KC * DL], F32, isOutput=False)
    wo_d = nc.declare_dram_parameter("woT", [DL, D], F32, isOutput=False)
    y_d = nc.declare_dram_parameter("y", [S, D], F32, isOutput=True)
    mask_d = None
    if is_causal:
        mask_d = nc.declare_dram_parameter("masks", [128, 128], F32, isOutput=False)
    bqk_d = None
    if with_bqk:
        # rows 0..DL-1 = bq, rows DL..2DL-1 = bk (per-partition bias columns)
        bqk_d = nc.declare_dram_parameter("bqk", [2 * DL, 1], F32, isOutput=False)
    bv_d = None
    if with_bv:
        bv_d = nc.declare_dram_parameter("bvb", [128, DL], F32, isOutput=False)

    with TileContext(nc) as tc:
        with (
            tc.tile_pool(name="const", bufs=1) as cp,
            tc.tile_pool(name="stream", bufs=3) as sp,
            tc.tile_pool(name="exps", bufs=8) as ep,
            tc.tile_pool(name="yout", bufs=3) as yp,
            tc.tile_pool(name="small", bufs=3) as smp,
        ):
            # ---- static loads -------------------------------------------------
            wq_sb = cp.tile([128, KC, DL], F32R, tag="wq", name="wq")
            wk_sb = cp.tile([128, KC, DL], F32R, tag="wk", name="wk")
            wv_sb = cp.tile([128, KC, DL], F32R, tag="wv", name="wv")
            for half in range(2):
                cols = slice(half * KC * DL // 2, (half + 1) * KC * DL // 2)
                nc.sync.dma_start(
                    out=wk_sb[:, half * KC // 2 : (half + 1) * KC // 2, :],
                    in_=wk_d[:, cols].bitcast(F32R),
                )
            mask_sb = None
            if is_causal:
                mask_sb = cp.tile([128, 128], F32R, tag="mask", name="mask")
            wo_sb = [cp.tile([128, D], F32R, tag=f"wo{c}", name=f"wo{c}") for c in range(2)]
            bqk_sb = None
            if with_bqk:
                bqk_sb = cp.tile([128, 4], F32, tag="bqk", name="bqk")
                nc.sync.dma_start(
                    out=bqk_sb[:], in_=bqk_d[:].rearrange("(c p) o -> p (c o)", p=128)
                )
            bv_sb = None
            if with_bv:
                bv_sb = cp.tile([128, DL], F32, tag="bvb", name="bvb")
                nc.sync.dma_start(out=bv_sb[:], in_=bv_d[:])

            # persistent activation tensors (feature-major)
            qT = [cp.tile([128, S], F32R, tag=f"qT{i}", name=f"qT{i}") for i in range(2)]
            kT = [cp.tile([128, S], F32R, tag=f"kT{i}", name=f"kT{i}") for i in range(2)]
            # V augmented with a ones column per head: [seq-tile, head, 65]
            v_aug = cp.tile([128, NKT, HL, HD + 1], F32R, tag="vaug", name="vaug")
            oT = [cp.tile([128, S], F32R, tag=f"oT{i}", name=f"oT{i}") for i in range(2)]
            ones_f32 = cp.tile([128, NKT * HL], F32, tag="ones", name="ones")
            nc.vector.memset(ones_f32[:], 1.0)
            nc.vector.tensor_copy(
                v_aug[:, :, :, HD : HD + 1],
                ones_f32[:].rearrange("p (a b) -> p a b", a=NKT)[:, :, :, None],
            )

            for rep in range(repeat):
                with tc.tile_pool(name=f"ppsum{rep}", bufs=1, space="PSUM") as pp:

                    def emit_proj(qb):
                        # ---- K^T / Q^T projection for this q block -----------
                        # out[m,n] = sum_k wT[k, m-dims] * xT[k, n-seq]
                        for pname, x_d, w_sb, out_tiles, bias_col in (
                            ("k", xk_d, wk_sb, kT, 1),
                            ("q", xq_d, wq_sb, qT, 0),
                        ):
                            if rep == 0 and qb == 0 and pname == "q":
                                nc.sync.dma_start(
                                    out=wq_sb[:].rearrange("p k d -> p (k d)"),
                                    in_=wq_d[:].bitcast(F32R),
                                )
                            xcb = sp.tile([128, KC, QB], F32R, tag="xqk", name="xqk", bufs=3)
                            blk = KC * QB
                            nsplit = 4 if (qb == 0 and rep == 0) else 2
                            for part in range(nsplit):
                                hk = KC // nsplit
                                nc.sync.dma_start(
                                    out=xcb[:, hk * part : hk * (part + 1), :],
                                    in_=x_d[
                                        :,
                                        qb * blk + part * blk // nsplit : qb * blk
                                        + (part + 1) * blk // nsplit,
                                    ].bitcast(F32R),
                                )
                            for m in range(2):
                                pk = pp.tile([128, QB], F32, tag="pp", name="pp", bufs=2)
                                for kc in range(KC):
                                    nc.tensor.matmul(
                                        pk[:],
                                        w_sb[:, kc, 128 * m : 128 * (m + 1)],
                                        xcb[:, kc, :],
                                        start=(kc == 0),
                                        stop=(kc == KC - 1),
                                    )
                                dst = out_tiles[m][:, QB * qb : QB * (qb + 1)]
                                if with_bqk:
                                    nc.scalar.activation(
                                        dst,
                                        pk[:],
                                        mybir.ActivationFunctionType.Identity,
                                        bias=bqk_sb[
                                            :, 2 * bias_col + m : 2 * bias_col + m + 1
                                        ],
                                    )
                                else:
                                    nc.vector.tensor_copy(dst, pk[:])

                        # ---- V projection for this stage's 4 seq tiles -------
                        if rep == 0 and qb == 0:
                            nc.sync.dma_start(
                                out=wv_sb[:].rearrange("p k d -> p (k d)"),
                                in_=wv_d[:].bitcast(F32R),
                            )
                        for qi in range(4):
                            qt = 4 * qb + qi
                            xcol = sp.tile(
                                [128, KC, 128], F32R, tag="xcol", name="xcol", bufs=4
                            )
                            nc.sync.dma_start(
                                out=xcol[:].rearrange("p k c -> p (k c)"),
                                in_=xv_d[
                                    :, qt * KC * 128 : (qt + 1) * KC * 128
                                ].bitcast(F32R),
                            )
                            pv = pp.tile([128, DL], F32, tag="pp", name="pv", bufs=2)
                            for kc in range(KC):
                                nc.tensor.matmul(
                                    pv[:],
                                    xcol[:, kc, :],
                                    wv_sb[:, kc, :],
                                    start=(kc == 0),
                                    stop=(kc == KC - 1),
                                )
                            vsrc = pv[:].rearrange("p (h d) -> p h d", h=HL)
                            vdst = v_aug[:, qt, :, 0:HD]
                            if with_bv:
                                nc.vector.tensor_add(
                                    vdst,
                                    vsrc,
                                    bv_sb[:]
                                    .rearrange("p (h d) -> p h d", h=HL)
                                    .bitcast(F32R),
                                )
                            else:
                                nc.vector.tensor_copy(vdst, vsrc)


                    def emit_attn(qb):
                        # ---- attention for this q block ----------------------
                        # Head pair (2*ht, 2*ht+1) computed concurrently on PE
                        # row groups (0,0)/(64,0).  Diagonal kk tiles narrow to
                        # the allowed q range; only the boundary 128-col
                        # blockette needs the triangle mask.
                        ntk = 4 * qb + 4 if is_causal else NKT
                        if rep == 0 and qb == 0:
                            if is_causal:
                                nc.sync.dma_start(
                                    out=mask_sb[:], in_=mask_d[:].bitcast(F32R)
                                )
                            for c in range(2):
                                nc.sync.dma_start(
                                    out=wo_sb[c][:],
                                    in_=wo_d[128 * c : 128 * (c + 1), :].bitcast(F32R),
                                )
                        for ht in range(2):
                            po_t = [
                                pp.tile(
                                    [HD + 1, QB], F32, tag="po", name="po", bufs=2
                                )
                                for _ in range(2)
                            ]
                            for t in range(ntk):
                                qlo = max(0, 128 * (t - 4 * qb)) if is_causal else 0
                                ps2 = [
                                    pp.tile([128, QB], F32, tag="ps", name="ps", bufs=4)
                                    for _ in range(2)
                                ]
                                es2 = []
                                for sub in range(2):
                                    hr = 64 * sub
                                    nc.tensor.matmul(
                                        ps2[sub][:, qlo:QB],
                                        kT[ht][hr : hr + 64, 128 * t : 128 * (t + 1)],
                                        qT[ht][hr : hr + 64, QB * qb + qlo : QB * (qb + 1)],
                                        start=True,
                                        stop=True,
                                        tile_position=(hr, 0),
                                    )
                                    es = ep.tile([128, QB], F32R, tag="es", name="es")
                                    nc.scalar.activation(
                                        es[:, qlo:QB],
                                        ps2[sub][:, qlo:QB],
                                        mybir.ActivationFunctionType.Exp,
                                        scale=SCALE,
                                    )
                                    if is_causal and t >= 4 * qb:
                                        nc.vector.tensor_mul(
                                            es[:, qlo : qlo + 128],
                                            es[:, qlo : qlo + 128],
                                            mask_sb[:],
                                        )
                                    es2.append(es)
                                for sub in range(2):
                                    h = 2 * ht + sub
                                    nc.tensor.matmul(
                                        po_t[sub][:, qlo:QB],
                                        v_aug[:, t, h, :],
                                        es2[sub][:, qlo:QB],
                                        start=(t == 0),
                                        stop=(t == ntk - 1),
                                    )
                            # rows 0..63 are O^T, row 64 is the softmax sum
                            for sub in range(2):
                                hr = 64 * sub
                                r = smp.tile([1, QB], F32, tag="r", name="r")
                                nc.vector.reciprocal(r[:], po_t[sub][HD : HD + 1, :])
                                rb = smp.tile([64, QB], F32, tag="rb", name="rb")
                                nc.gpsimd.partition_broadcast(rb[:], r[0:1, :])
                                nc.vector.tensor_mul(
                                    oT[ht][hr : hr + 64, QB * qb : QB * (qb + 1)],
                                    po_t[sub][0:HD, :],
                                    rb[:],
                                )
                        # ---- output projection for this q block --------------
                        for qi in range(4):
                            qt = 4 * qb + qi
                            yt = yp.tile([128, D], F32, tag="yt", name="yt")
                            for nb2 in range(2):
                                py = pp.tile([128, QB], F32, tag="ps", name="py", bufs=4)
                                for c in range(2):
                                    nc.tensor.matmul(
                                        py[:],
                                        oT[c][:, 128 * qt : 128 * (qt + 1)],
                                        wo_sb[c][:, 512 * nb2 : 512 * (nb2 + 1)],
                                        start=(c == 0),
                                        stop=(c == 1),
                                    )
                                nc.vector.tensor_copy(
                                    yt[:, 512 * nb2 : 512 * (nb2 + 1)], py[:]
                                )
                            nc.scalar.dma_start(
                                out=y_d[128 * qt : 128 * (qt + 1), :], in_=yt[:]
                            )



                    if is_causal:
                        # streaming: attention(qb) only needs K/V up to qb
                        for qb in range(NQB):
                            emit_proj(qb)
                            emit_attn(qb)
                    else:
                        # attention needs the full K/V: project everything first
                        for qb in range(NQB):
                            emit_proj(qb)
                        for qb in range(NQB):
                            emit_attn(qb)
    nc.finalize()
    return nc


def _get_program(is_causal, with_bqk, with_bv, repeat=1):
    key = (bool(is_causal), bool(with_bqk), bool(with_bv), repeat)
    if key not in _prog_cache:
        _prog_cache[key] = _build(*key)
    return _prog_cache[key]


def _make_masks():
    i = np.arange(128)[:, None]
    j = np.arange(128)[None, :]
    return (j >= i).astype(np.float32)


def _make_in_maps(Q_in, K_in, V_in, Wq, bq, Wk, bk, Wv, bv, Wo, bo, is_causal):
    with_bqk = bool(np.any(bq) or np.any(bk))
    with_bv = bool(np.any(bv))
    masks = _make_masks() if is_causal else None

    def shuf_qk(x):
        # X^T [(k p), (qb s)] -> [p, (qb k s)] so each stage DMA is contiguous
        return np.ascontiguousarray(
            x.T.reshape(KC, 128, NQB, QB).transpose(1, 2, 0, 3).reshape(128, -1)
        )

    def shuf_v(x):
        # X^T [(k p), (qt c)] -> [p, (qt k c)]
        return np.ascontiguousarray(
            x.T.reshape(KC, 128, NQT, 128).transpose(1, 2, 0, 3).reshape(128, -1)
        )

    def shuf_w(w):
        # W slice [dl, D] -> W^T [(k p), dl] -> [p, (k dl)]
        return np.ascontiguousarray(
            w.T.reshape(KC, 128, DL).transpose(1, 0, 2).reshape(128, -1)
        )

    xT = {}
    for b in range(BG):
        xT[("q", b)] = shuf_qk(Q_in[b])
        xT[("k", b)] = shuf_qk(K_in[b])
        xT[("v", b)] = shuf_v(V_in[b])
    in_maps = []
    for core in range(NCORES):
        b, hg = core // HG, core % HG
        sl = slice(DL * hg, DL * (hg + 1))
        m = {
            "xqT": xT[("q", b)],
            "xkT": xT[("k", b)],
            "xvT": xT[("v", b)],
            "wqT": shuf_w(Wq[sl, :]),
            "wkT": shuf_w(Wk[sl, :]),
            "wvT": shuf_w(Wv[sl, :]),
            "woT": np.ascontiguousarray(Wo[:, sl].T),
        }
        if is_causal:
            m["masks"] = masks
        if with_bqk:
            m["bqk"] = np.concatenate([bq[sl], bk[sl]]).astype(np.float32)[:, None]
        if with_bv:
            m["bvb"] = np.broadcast_to(bv[sl], (128, DL)).astype(np.float32).copy()
        in_maps.append(m)
    return in_maps, with_bqk, with_bv


def kernel(Q_in, K_in, V_in, Wq, bq, Wk, bk, Wv, bv, Wo, bo, is_causal):
    Q_in, K_in, V_in = (np.asarray(a, np.float32) for a in (Q_in, K_in, V_in))
    Wq, Wk, Wv, Wo = (np.asarray(a, np.float32) for a in (Wq, Wk, Wv, Wo))
    bq, bk, bv, bo = (np.asarray(a, np.float32) for a in (bq, bk, bv, bo))
    causal = bool(int(np.asarray(is_causal)))

    in_maps, with_bqk, with_bv = _make_in_maps(
        Q_in, K_in, V_in, Wq, bq, Wk, bk, Wv, bv, Wo, bo, causal
    )
    nc = _get_program(causal, with_bqk, with_bv)
    res = run_bass_kernel_spmd(nc, in_maps, list(range(NCORES)))
    out = np.zeros((B, S, D), np.float32)
    for core in range(NCORES):
        out[core // HG] += res.results[core]["y"]
    out += bo
    return out



# revision 2
# speedup vs baseline: 1.8253x; 1.8253x over previous
"""Trainium2 Bass kernel for nn_MultiHeadAttention (B=2, S=2048, D=1024, H=16).

Sharding: 8 cores = 2 batch groups x 4 head groups (4 heads / core).
Host pre-transposes X and the weight slices (and converts them to bf16) so the
device kernel needs no on-chip transposes:
  - activations flow feature-major (Q^T, K^T [256, 2048]) through scores,
  - scores are computed transposed (S^T[kk, q]) so softmax-sum comes from a
    ones-column augmented into V via the PV matmul itself,
  - head pairs share one 2-bank PSUM score tile so each exp activation covers
    both heads; causal boundary masks run on the otherwise-idle gpsimd queue,
  - emission order interleaves proj(qb+1) between attention(qb) and the
    output projection(qb) so the softmax-normalize latency hides under PE work,
  - the output projection flips back to seq-major [2048, 1024] fp32 partials,
  - host sums the 4 head-group partials per batch and adds the output bias.
All matmuls run in bf16 (full PE rate); accumulation stays fp32 in PSUM.
"""

import os
import sys

for _p in ("/opt/trn_rl_repo",):
    if os.path.isdir(_p) and _p not in sys.path:
        sys.path.append(_p)

import numpy as np

import concourse.bacc as bacc
import concourse.mybir as mybir
from concourse.bass_utils import run_bass_kernel_spmd
from concourse.tile import TileContext

F32 = mybir.dt.float32
BF16 = mybir.dt.bfloat16

B, S, D, H, HD = 2, 2048, 1024, 16, 64
NCORES = 8
BG = 2                 # batch groups
HG = NCORES // BG      # head groups (cores per batch)
HL = H // HG           # heads per core = 4
DL = HL * HD           # local head dims = 256
KC = D // 128          # contraction chunks for the projections = 8
QB = 512               # q block (matmul free dim)
NQB = S // QB          # 4
KT = 128               # key tile (scores partition dim)
NKT = S // KT          # 16
QT = 128               # out-proj q tile
NQT = S // QT          # 16
SCALE = 1.0 / np.sqrt(HD)

_prog_cache = {}


def _build(is_causal, with_bqk, with_bv, repeat=1):
    nc = bacc.Bacc(None, target_bir_lowering=False, debug=False)

    xq_d = nc.declare_dram_parameter("xqT", [128, NQB * KC * QB], BF16, isOutput=False)
    xk_d = nc.declare_dram_parameter("xkT", [128, NQB * KC * QB], BF16, isOutput=False)
    xv_d = nc.declare_dram_parameter("xvT", [128, NQT * KC * 128], BF16, isOutput=False)
    wq_d = nc.declare_dram_parameter("wqT", [128, KC * DL], BF16, isOutput=False)
    wk_d = nc.declare_dram_parameter("wkT", [128, KC * DL], BF16, isOutput=False)
    wv_d = nc.declare_dram_parameter("wvT", [128, KC * DL], BF16, isOutput=False)
    wo_d = nc.declare_dram_parameter("woT", [DL, D], BF16, isOutput=False)
    y_d = nc.declare_dram_parameter("y", [S, D], F32, isOutput=True)
    mask_d = None
    if is_causal:
        # triangle mask duplicated for the two PE row groups: [128, 2, 128]
        mask_d = nc.declare_dram_parameter("masks", [128, 2 * 128], BF16, isOutput=False)
    bqk_d = None
    if with_bqk:
        # rows 0..DL-1 = bq, rows DL..2DL-1 = bk (per-partition bias columns)
        bqk_d = nc.declare_dram_parameter("bqk", [2 * DL, 1], F32, isOutput=False)
    bv_d = None
    if with_bv:
        bv_d = nc.declare_dram_parameter("bvb", [128, DL], F32, isOutput=False)

    with TileContext(nc) as tc:
        with (
            tc.tile_pool(name="const", bufs=1) as cp,
            tc.tile_pool(name="stream", bufs=3) as sp,
            tc.tile_pool(name="exps", bufs=4) as ep,
            tc.tile_pool(name="yout", bufs=3) as yp,
            tc.tile_pool(name="small", bufs=3) as smp,
        ):
            # ---- static loads -------------------------------------------------
            wq_sb = cp.tile([128, KC, DL], BF16, tag="wq", name="wq")
            wk_sb = cp.tile([128, KC, DL], BF16, tag="wk", name="wk")
            wv_sb = cp.tile([128, KC, DL], BF16, tag="wv", name="wv")
            for half in range(2):
                cols = slice(half * KC * DL // 2, (half + 1) * KC * DL // 2)
                nc.sync.dma_start(
                    out=wk_sb[:, half * KC // 2 : (half + 1) * KC // 2, :],
                    in_=wk_d[:, cols],
                )
            mask_sb = None
            if is_causal:
                mask_sb = cp.tile([128, 2, 128], BF16, tag="mask", name="mask")
            wo_sb = [cp.tile([128, D], BF16, tag=f"wo{c}", name=f"wo{c}") for c in range(2)]
            bqk_sb = None
            if with_bqk:
                bqk_sb = cp.tile([128, 4], F32, tag="bqk", name="bqk")
                nc.sync.dma_start(
                    out=bqk_sb[:], in_=bqk_d[:].rearrange("(c p) o -> p (c o)", p=128)
                )
            bv_sb = None
            if with_bv:
                bv_sb = cp.tile([128, DL], F32, tag="bvb", name="bvb")
                nc.sync.dma_start(out=bv_sb[:], in_=bv_d[:])

            # persistent activation tensors (feature-major)
            qT = [cp.tile([128, S], BF16, tag=f"qT{i}", name=f"qT{i}") for i in range(2)]
            kT = [cp.tile([128, S], BF16, tag=f"kT{i}", name=f"kT{i}") for i in range(2)]
            # V augmented with a ones column per head: [seq-tile, head, 65]
            v_aug = cp.tile([128, NKT, HL, HD + 1], BF16, tag="vaug", name="vaug")
            oT = [cp.tile([128, S], BF16, tag=f"oT{i}", name=f"oT{i}") for i in range(2)]
            ones_bf = cp.tile([128, NKT * HL], BF16, tag="ones", name="ones")
            nc.vector.memset(ones_bf[:], 1.0)
            nc.vector.tensor_copy(
                v_aug[:, :, :, HD : HD + 1],
                ones_bf[:].rearrange("p (a b) -> p a b", a=NKT)[:, :, :, None],
            )

            for rep in range(repeat):
                with tc.tile_pool(name=f"ppsum{rep}", bufs=1, space="PSUM") as pp:

                    def emit_proj(qb):
                        # ---- K^T / Q^T projection for this q block -----------
                        # out[m,n] = sum_k wT[k, m-dims] * xT[k, n-seq]
                        for pname, x_d, w_sb, out_tiles, bias_col in (
                            ("k", xk_d, wk_sb, kT, 1),
                            ("q", xq_d, wq_sb, qT, 0),
                        ):
                            if rep == 0 and qb == 0 and pname == "q":
                                nc.sync.dma_start(
                                    out=wq_sb[:].rearrange("p k d -> p (k d)"),
                                    in_=wq_d[:],
                                )
                            xcb = sp.tile([128, KC, QB], BF16, tag="xqk", name="xqk", bufs=3)
                            blk = KC * QB
                            nsplit = 4 if (qb == 0 and rep == 0) else 2
                            for part in range(nsplit):
                                hk = KC // nsplit
                                nc.sync.dma_start(
                                    out=xcb[:, hk * part : hk * (part + 1), :],
                                    in_=x_d[
                                        :,
                                        qb * blk + part * blk // nsplit : qb * blk
                                        + (part + 1) * blk // nsplit,
                                    ],
                                )
                            for m in range(2):
                                pk = pp.tile([128, QB], F32, tag="pp", name="pp", bufs=2)
                                for kc in range(KC):
                                    nc.tensor.matmul(
                                        pk[:],
                                        w_sb[:, kc, 128 * m : 128 * (m + 1)],
                                        xcb[:, kc, :],
                                        start=(kc == 0),
                                        stop=(kc == KC - 1),
                                    )
                                dst = out_tiles[m][:, QB * qb : QB * (qb + 1)]
                                if with_bqk:
                                    nc.scalar.activation(
                                        dst,
                                        pk[:],
                                        mybir.ActivationFunctionType.Identity,
                                        bias=bqk_sb[
                                            :, 2 * bias_col + m : 2 * bias_col + m + 1
                                        ],
                                    )
                                else:
                                    nc.vector.tensor_copy(dst, pk[:])

                        # ---- V projection for this stage's 4 seq tiles -------
                        if rep == 0 and qb == 0:
                            nc.sync.dma_start(
                                out=wv_sb[:].rearrange("p k d -> p (k d)"),
                                in_=wv_d[:],
                            )
                        for qi in range(4):
                            qt = 4 * qb + qi
                            xcol = sp.tile(
                                [128, KC, 128], BF16, tag="xcol", name="xcol", bufs=4
                            )
                            nc.sync.dma_start(
                                out=xcol[:].rearrange("p k c -> p (k c)"),
                                in_=xv_d[:, qt * KC * 128 : (qt + 1) * KC * 128],
                            )
                            pv = pp.tile([128, DL], F32, tag="pp", name="pv", bufs=2)
                            for kc in range(KC):
                                nc.tensor.matmul(
                                    pv[:],
                                    xcol[:, kc, :],
                                    wv_sb[:, kc, :],
                                    start=(kc == 0),
                                    stop=(kc == KC - 1),
                                )
                            vsrc = pv[:].rearrange("p (h d) -> p h d", h=HL)
                            vdst = v_aug[:, qt, :, 0:HD]
                            if with_bv:
                                nc.vector.tensor_add(
                                    vdst,
                                    vsrc,
                                    bv_sb[:].rearrange("p (h d) -> p h d", h=HL),
                                )
                            else:
                                nc.vector.tensor_copy(vdst, vsrc)

                    def emit_attn(qb):
                        # ---- attention for this q block ----------------------
                        # Head pair (2*ht, 2*ht+1) computed concurrently on PE
                        # row groups (0,0)/(64,0) into one 2-bank PSUM tile so
                        # a single exp covers both.  Diagonal kk tiles narrow
                        # to the allowed q range; only the boundary 128-col
                        # blockette needs the triangle mask (on gpsimd).
                        ntk = 4 * qb + 4 if is_causal else NKT
                        if rep == 0 and qb == 0:
                            if is_causal:
                                nc.sync.dma_start(
                                    out=mask_sb[:].rearrange("p a b -> p (a b)"),
                                    in_=mask_d[:],
                                )
                            for c in range(2):
                                nc.sync.dma_start(
                                    out=wo_sb[c][:],
                                    in_=wo_d[128 * c : 128 * (c + 1), :],
                                )
                        for ht in range(2):
                            po_t = [
                                pp.tile(
                                    [HD + 1, QB], F32, tag="po", name="po", bufs=2
                                )
                                for _ in range(2)
                            ]
                            for t in range(ntk):
                                qlo = max(0, 128 * (t - 4 * qb)) if is_causal else 0
                                ps = pp.tile(
                                    [128, 2, QB], F32, tag="ps", name="ps", bufs=2
                                )
                                for sub in range(2):
                                    hr = 64 * sub
                                    nc.tensor.matmul(
                                        ps[:, sub, qlo:QB],
                                        kT[ht][hr : hr + 64, 128 * t : 128 * (t + 1)],
                                        qT[ht][hr : hr + 64, QB * qb + qlo : QB * (qb + 1)],
                                        start=True,
                                        stop=True,
                                        tile_position=(hr, 0),
                                    )
                                es = ep.tile([128, 2, QB], BF16, tag="es", name="es")
                                nc.scalar.activation(
                                    es[:, :, qlo:QB],
                                    ps[:, :, qlo:QB],
                                    mybir.ActivationFunctionType.Exp,
                                    scale=SCALE,
                                )
                                if is_causal and t >= 4 * qb:
                                    nc.gpsimd.tensor_mul(
                                        es[:, :, qlo : qlo + 128],
                                        es[:, :, qlo : qlo + 128],
                                        mask_sb[:],
                                    )
                                for sub in range(2):
                                    h = 2 * ht + sub
                                    nc.tensor.matmul(
                                        po_t[sub][:, qlo:QB],
                                        v_aug[:, t, h, :],
                                        es[:, sub, qlo:QB],
                                        start=(t == 0),
                                        stop=(t == ntk - 1),
                                    )
                            # rows 0..63 are O^T, row 64 is the softmax sum
                            for sub in range(2):
                                hr = 64 * sub
                                r = smp.tile([1, QB], F32, tag="r", name="r")
                                nc.vector.reciprocal(r[:], po_t[sub][HD : HD + 1, :])
                                rb = smp.tile([64, QB], F32, tag="rb", name="rb")
                                nc.gpsimd.partition_broadcast(rb[:], r[0:1, :])
                                nc.vector.tensor_mul(
                                    oT[ht][hr : hr + 64, QB * qb : QB * (qb + 1)],
                                    po_t[sub][0:HD, :],
                                    rb[:],
                                )

                    def emit_outproj(qb):
                        # ---- output projection for this q block --------------
                        for qi in range(4):
                            qt = 4 * qb + qi
                            yt = yp.tile([128, D], F32, tag="yt", name="yt")
                            for nb2 in range(2):
                                py = pp.tile([128, QB], F32, tag="pp", name="py", bufs=2)
                                for c in range(2):
                                    nc.tensor.matmul(
                                        py[:],
                                        oT[c][:, 128 * qt : 128 * (qt + 1)],
                                        wo_sb[c][:, 512 * nb2 : 512 * (nb2 + 1)],
                                        start=(c == 0),
                                        stop=(c == 1),
                                    )
                                nc.vector.tensor_copy(
                                    yt[:, 512 * nb2 : 512 * (nb2 + 1)], py[:]
                                )
                            nc.sync.dma_start(
                                out=y_d[128 * qt : 128 * (qt + 1), :], in_=yt[:]
                            )

                    if is_causal:
                        # streaming: attention(qb) only needs K/V up to qb.
                        # proj(qb+1) is emitted between attn(qb) and
                        # outproj(qb) so the PE queue has independent work
                        # while the softmax-normalize chain completes.
                        emit_proj(0)
                        for qb in range(NQB):
                            emit_attn(qb)
                            if qb + 1 < NQB:
                                emit_proj(qb + 1)
                            emit_outproj(qb)
                    else:
                        # attention needs the full K/V: project everything first
                        for qb in range(NQB):
                            emit_proj(qb)
                        for qb in range(NQB):
                            emit_attn(qb)
                            emit_outproj(qb)
    nc.finalize()
    return nc


def _get_program(is_causal, with_bqk, with_bv, repeat=1):
    key = (bool(is_causal), bool(with_bqk), bool(with_bv), repeat)
    if key not in _prog_cache:
        _prog_cache[key] = _build(*key)
    return _prog_cache[key]


def _bf16(x):
    import ml_dtypes

    return np.ascontiguousarray(x).astype(ml_dtypes.bfloat16)


def _make_masks():
    i = np.arange(128)[:, None]
    j = np.arange(128)[None, :]
    tri = (j >= i).astype(np.float32)
    return np.concatenate([tri, tri], axis=1)  # [128, 2*128]


def _make_in_maps(Q_in, K_in, V_in, Wq, bq, Wk, bk, Wv, bv, Wo, bo, is_causal):
    with_bqk = bool(np.any(bq) or np.any(bk))
    with_bv = bool(np.any(bv))
    masks = _bf16(_make_masks()) if is_causal else None

    def shuf_qk(x):
        # X^T [(k p), (qb s)] -> [p, (qb k s)] so each stage DMA is contiguous
        return _bf16(
            x.T.reshape(KC, 128, NQB, QB).transpose(1, 2, 0, 3).reshape(128, -1)
        )

    def shuf_v(x):
        # X^T [(k p), (qt c)] -> [p, (qt k c)]
        return _bf16(
            x.T.reshape(KC, 128, NQT, 128).transpose(1, 2, 0, 3).reshape(128, -1)
        )

    def shuf_w(w):
        # W slice [dl, D] -> W^T [(k p), dl] -> [p, (k dl)]
        return _bf16(
            w.T.reshape(KC, 128, DL).transpose(1, 0, 2).reshape(128, -1)
        )

    xT = {}
    for b in range(BG):
        xT[("q", b)] = shuf_qk(Q_in[b])
        xT[("k", b)] = shuf_qk(K_in[b])
        xT[("v", b)] = shuf_v(V_in[b])
    in_maps = []
    for core in range(NCORES):
        b, hg = core // HG, core % HG
        sl = slice(DL * hg, DL * (hg + 1))
        m = {
            "xqT": xT[("q", b)],
            "xkT": xT[("k", b)],
            "xvT": xT[("v", b)],
            "wqT": shuf_w(Wq[sl, :]),
            "wkT": shuf_w(Wk[sl, :]),
            "wvT": shuf_w(Wv[sl, :]),
            "woT": _bf16(Wo[:, sl].T),
        }
        if is_causal:
            m["masks"] = masks
        if with_bqk:
            m["bqk"] = np.concatenate([bq[sl], bk[sl]]).astype(np.float32)[:, None]
        if with_bv:
            m["bvb"] = np.broadcast_to(bv[sl], (128, DL)).astype(np.float32).copy()
        in_maps.append(m)
    return in_maps, with_bqk, with_bv


def kernel(Q_in, K_in, V_in, Wq, bq, Wk, bk, Wv, bv, Wo, bo, is_causal):
    Q_in, K_in, V_in = (np.asarray(a, np.float32) for a in (Q_in, K_in, V_in))
    Wq, Wk, Wv, Wo = (np.asarray(a, np.float32) for a in (Wq, Wk, Wv, Wo))
    bq, bk, bv, bo = (np.asarray(a, np.float32) for a in (bq, bk, bv, bo))
    causal = bool(int(np.asarray(is_causal)))

    in_maps, with_bqk, with_bv = _make_in_maps(
        Q_in, K_in, V_in, Wq, bq, Wk, bk, Wv, bv, Wo, bo, causal
    )
    nc = _get_program(causal, with_bqk, with_bv)
    res = run_bass_kernel_spmd(nc, in_maps, list(range(NCORES)))
    out = np.zeros((B, S, D), np.float32)
    for core in range(NCORES):
        out[core // HG] += res.results[core]["y"]
    out += bo
    return out


# revision 5
# speedup vs baseline: 14.0704x; 7.7086x over previous
"""Trainium2 Bass kernel for nn_MultiHeadAttention (B=2, S=2048, D=1024, H=16).

Sharding: 8 cores = 2 batch groups x 4 head groups (4 heads / core).
Host pre-transposes X and the weight slices (and converts them to bf16) so the
device kernel needs no on-chip transposes:
  - activations flow feature-major (Q^T, K^T [256, 2048]) through scores,
  - scores are computed transposed (S^T[kk, q]) so softmax-sum comes from a
    ones-column augmented into V via the PV matmul itself,
  - head pairs share one 2-bank PSUM score tile so each exp activation covers
    both heads; causal boundary masks run on the otherwise-idle gpsimd queue,
  - emission order interleaves proj(qb+1) between attention(qb) and the
    output projection(qb) so the softmax-normalize latency hides under PE work,
  - the output projection flips back to seq-major [2048, 1024] fp32 partials,
  - host sums the 4 head-group partials per batch and adds the output bias.
All matmuls run in bf16 (full PE rate); accumulation stays fp32 in PSUM.
"""

import os
import sys

for _p in ("/opt/trn_rl_repo",):
    if os.path.isdir(_p) and _p not in sys.path:
        sys.path.append(_p)

import numpy as np

import concourse.bacc as bacc
import concourse.mybir as mybir
from concourse.bass_utils import run_bass_kernel_spmd
from concourse.tile import TileContext

F32 = mybir.dt.float32
BF16 = mybir.dt.bfloat16

B, S, D, H, HD = 2, 2048, 1024, 16, 64
NCORES = 8
BG = 2                 # batch groups
HG = NCORES // BG      # head groups (cores per batch)
HL = H // HG           # heads per core = 4
DL = HL * HD           # local head dims = 256
KC = D // 128          # contraction chunks for the projections = 8
QB = 512               # q block (matmul free dim)
NQB = S // QB          # 4
KT = 128               # key tile (scores partition dim)
NKT = S // KT          # 16
QT = 128               # out-proj q tile
NQT = S // QT          # 16
SCALE = 1.0 / np.sqrt(HD)

_prog_cache = {}


def _build(is_causal, with_bqk, with_bv, repeat=1):
    nc = bacc.Bacc(None, target_bir_lowering=False, debug=False)

    xq_d = nc.declare_dram_parameter("xqT", [128, NQB * KC * QB], BF16, isOutput=False)
    xk_d = nc.declare_dram_parameter("xkT", [128, NQB * KC * QB], BF16, isOutput=False)
    xv_d = nc.declare_dram_parameter("xvT", [128, NQT * KC * 128], BF16, isOutput=False)
    wq_d = nc.declare_dram_parameter("wqT", [128, KC * DL], BF16, isOutput=False)
    wk_d = nc.declare_dram_parameter("wkT", [128, KC * DL], BF16, isOutput=False)
    wv_d = nc.declare_dram_parameter("wvT", [128, KC * DL], BF16, isOutput=False)
    wo_d = nc.declare_dram_parameter("woT", [DL, D], BF16, isOutput=False)
    y_d = nc.declare_dram_parameter("y", [S, D], F32, isOutput=True)
    mask_d = None
    if is_causal:
        # triangle mask duplicated for the two PE row groups: [128, 2, 128]
        mask_d = nc.declare_dram_parameter("masks", [128, 2 * 128], BF16, isOutput=False)
    bqk_d = None
    if with_bqk:
        # rows 0..DL-1 = bq, rows DL..2DL-1 = bk (per-partition bias columns)
        bqk_d = nc.declare_dram_parameter("bqk", [2 * DL, 1], F32, isOutput=False)
    bv_d = None
    if with_bv:
        bv_d = nc.declare_dram_parameter("bvb", [128, DL], F32, isOutput=False)

    with TileContext(nc) as tc:
        with (
            tc.tile_pool(name="const", bufs=1) as cp,
            tc.tile_pool(name="stream", bufs=3) as sp,
            tc.tile_pool(name="exps", bufs=4) as ep,
            tc.tile_pool(name="yout", bufs=3) as yp,
            tc.tile_pool(name="small", bufs=3) as smp,
            tc.tile_pool(name="ppsum", bufs=1, space="PSUM") as pp,
        ):
            # ---- static loads -------------------------------------------------
            wq_sb = cp.tile([128, KC, DL], BF16, tag="wq", name="wq")
            wk_sb = cp.tile([128, KC, DL], BF16, tag="wk", name="wk")
            wv_sb = cp.tile([128, KC, DL], BF16, tag="wv", name="wv")
            for half in range(2):
                cols = slice(half * KC * DL // 2, (half + 1) * KC * DL // 2)
                nc.sync.dma_start(
                    out=wk_sb[:, half * KC // 2 : (half + 1) * KC // 2, :],
                    in_=wk_d[:, cols],
                )
            mask_sb = None
            if is_causal:
                mask_sb = cp.tile([128, 2, 128], BF16, tag="mask", name="mask")
            wo_sb = [cp.tile([128, D], BF16, tag=f"wo{c}", name=f"wo{c}") for c in range(2)]
            bqk_sb = None
            if with_bqk:
                bqk_sb = cp.tile([128, 4], F32, tag="bqk", name="bqk")
                nc.sync.dma_start(
                    out=bqk_sb[:], in_=bqk_d[:].rearrange("(c p) o -> p (c o)", p=128)
                )
            bv_sb = None
            if with_bv:
                bv_sb = cp.tile([128, DL], F32, tag="bvb", name="bvb")
                nc.sync.dma_start(out=bv_sb[:], in_=bv_d[:])

            # persistent activation tensors (feature-major)
            qT = [cp.tile([128, S], BF16, tag=f"qT{i}", name=f"qT{i}") for i in range(2)]
            kT = [cp.tile([128, S], BF16, tag=f"kT{i}", name=f"kT{i}") for i in range(2)]
            # V augmented with a ones column per head: [seq-tile, head, 65]
            v_aug = cp.tile([128, NKT, HL, HD + 1], BF16, tag="vaug", name="vaug")
            oT = [cp.tile([128, S], BF16, tag=f"oT{i}", name=f"oT{i}") for i in range(2)]
            ones_bf = cp.tile([128, NKT * HL], BF16, tag="ones", name="ones")
            nc.vector.memset(ones_bf[:], 1.0)
            nc.vector.tensor_copy(
                v_aug[:, :, :, HD : HD + 1],
                ones_bf[:].rearrange("p (a b) -> p a b", a=NKT)[:, :, :, None],
            )

            if True:
                if True:

                    def emit_proj(qb, load_w=False):
                        # ---- K^T / Q^T projection for this q block -----------
                        # out[m,n] = sum_k wT[k, m-dims] * xT[k, n-seq]
                        for pname, x_d, w_sb, out_tiles, bias_col in (
                            ("k", xk_d, wk_sb, kT, 1),
                            ("q", xq_d, wq_sb, qT, 0),
                        ):
                            if load_w and pname == "q":
                                nc.sync.dma_start(
                                    out=wq_sb[:].rearrange("p k d -> p (k d)"),
                                    in_=wq_d[:],
                                )
                            xcb = sp.tile([128, KC, QB], BF16, tag="xqk", name="xqk", bufs=3)
                            blk = KC * QB
                            nsplit = 4 if load_w else 2
                            for part in range(nsplit):
                                hk = KC // nsplit
                                nc.sync.dma_start(
                                    out=xcb[:, hk * part : hk * (part + 1), :],
                                    in_=x_d[
                                        :,
                                        qb * blk + part * blk // nsplit : qb * blk
                                        + (part + 1) * blk // nsplit,
                                    ],
                                )
                            for m in range(2):
                                pk = pp.tile([128, QB], F32, tag="pp", name="pp", bufs=2)
                                for kc in range(KC):
                                    nc.tensor.matmul(
                                        pk[:],
                                        w_sb[:, kc, 128 * m : 128 * (m + 1)],
                                        xcb[:, kc, :],
                                        start=(kc == 0),
                                        stop=(kc == KC - 1),
                                    )
                                dst = out_tiles[m][:, QB * qb : QB * (qb + 1)]
                                if with_bqk:
                                    nc.scalar.activation(
                                        dst,
                                        pk[:],
                                        mybir.ActivationFunctionType.Identity,
                                        bias=bqk_sb[
                                            :, 2 * bias_col + m : 2 * bias_col + m + 1
                                        ],
                                    )
                                else:
                                    nc.vector.tensor_copy(dst, pk[:])

                        # ---- V projection for this stage's 4 seq tiles -------
                        if load_w:
                            nc.sync.dma_start(
                                out=wv_sb[:].rearrange("p k d -> p (k d)"),
                                in_=wv_d[:],
                            )
                        for qi in range(4):
                            qt = 4 * qb + qi
                            xcol = sp.tile(
                                [128, KC, 128], BF16, tag="xcol", name="xcol", bufs=4
                            )
                            nc.sync.dma_start(
                                out=xcol[:].rearrange("p k c -> p (k c)"),
                                in_=xv_d[:, qt * KC * 128 : (qt + 1) * KC * 128],
                            )
                            pv = pp.tile([128, DL], F32, tag="pp", name="pv", bufs=2)
                            for kc in range(KC):
                                nc.tensor.matmul(
                                    pv[:],
                                    xcol[:, kc, :],
                                    wv_sb[:, kc, :],
                                    start=(kc == 0),
                                    stop=(kc == KC - 1),
                                )
                            vsrc = pv[:].rearrange("p (h d) -> p h d", h=HL)
                            vdst = v_aug[:, qt, :, 0:HD]
                            if with_bv:
                                nc.vector.tensor_add(
                                    vdst,
                                    vsrc,
                                    bv_sb[:].rearrange("p (h d) -> p h d", h=HL),
                                )
                            else:
                                nc.vector.tensor_copy(vdst, vsrc)

                    def emit_attn(qb, load_w=False):
                        # ---- attention for this q block ----------------------
                        # Head pair (2*ht, 2*ht+1) computed concurrently on PE
                        # row groups (0,0)/(64,0) into one 2-bank PSUM tile so
                        # a single exp covers both.  Diagonal kk tiles narrow
                        # to the allowed q range; only the boundary 128-col
                        # blockette needs the triangle mask (on gpsimd).
                        ntk = 4 * qb + 4 if is_causal else NKT
                        if load_w:
                            if is_causal:
                                nc.sync.dma_start(
                                    out=mask_sb[:].rearrange("p a b -> p (a b)"),
                                    in_=mask_d[:],
                                )
                            for c in range(2):
                                nc.sync.dma_start(
                                    out=wo_sb[c][:],
                                    in_=wo_d[128 * c : 128 * (c + 1), :],
                                )
                        for ht in range(2):
                            po_t = [
                                pp.tile(
                                    [HD + 1, QB], F32, tag="po", name="po", bufs=2
                                )
                                for _ in range(2)
                            ]
                            for t in range(ntk):
                                qlo = max(0, 128 * (t - 4 * qb)) if is_causal else 0
                                ps = pp.tile(
                                    [128, 2, QB], F32, tag="ps", name="ps", bufs=2
                                )
                                for sub in range(2):
                                    hr = 64 * sub
                                    nc.tensor.matmul(
                                        ps[:, sub, qlo:QB],
                                        kT[ht][hr : hr + 64, 128 * t : 128 * (t + 1)],
                                        qT[ht][hr : hr + 64, QB * qb + qlo : QB * (qb + 1)],
                                        start=True,
                                        stop=True,
                                        tile_position=(hr, 0),
                                    )
                                es = ep.tile([128, 2, QB], BF16, tag="es", name="es")
                                nc.scalar.activation(
                                    es[:, :, qlo:QB],
                                    ps[:, :, qlo:QB],
                                    mybir.ActivationFunctionType.Exp,
                                    scale=SCALE,
                                )
                                if is_causal and t >= 4 * qb:
                                    nc.gpsimd.tensor_mul(
                                        es[:, :, qlo : qlo + 128],
                                        es[:, :, qlo : qlo + 128],
                                        mask_sb[:],
                                    )
                                for sub in range(2):
                                    h = 2 * ht + sub
                                    nc.tensor.matmul(
                                        po_t[sub][:, qlo:QB],
                                        v_aug[:, t, h, :],
                                        es[:, sub, qlo:QB],
                                        start=(t == 0),
                                        stop=(t == ntk - 1),
                                    )
                            # rows 0..63 are O^T, row 64 is the softmax sum
                            for sub in range(2):
                                hr = 64 * sub
                                r = smp.tile([1, QB], F32, tag="r", name="r")
                                nc.vector.reciprocal(r[:], po_t[sub][HD : HD + 1, :])
                                rb = smp.tile([64, QB], F32, tag="rb", name="rb")
                                nc.gpsimd.partition_broadcast(rb[:], r[0:1, :])
                                nc.vector.tensor_mul(
                                    oT[ht][hr : hr + 64, QB * qb : QB * (qb + 1)],
                                    po_t[sub][0:HD, :],
                                    rb[:],
                                )

                    def emit_outproj(qb):
                        # ---- output projection for this q block --------------
                        for qi in range(4):
                            qt = 4 * qb + qi
                            yt = yp.tile([128, D], F32, tag="yt", name="yt")
                            for nb2 in range(2):
                                py = pp.tile([128, QB], F32, tag="pp", name="py", bufs=2)
                                for c in range(2):
                                    nc.tensor.matmul(
                                        py[:],
                                        oT[c][:, 128 * qt : 128 * (qt + 1)],
                                        wo_sb[c][:, 512 * nb2 : 512 * (nb2 + 1)],
                                        start=(c == 0),
                                        stop=(c == 1),
                                    )
                                nc.vector.tensor_copy(
                                    yt[:, 512 * nb2 : 512 * (nb2 + 1)], py[:]
                                )
                            nc.sync.dma_start(
                                out=y_d[128 * qt : 128 * (qt + 1), :], in_=yt[:]
                            )

                    if is_causal:
                        # streaming: attention(qb) only needs K/V up to qb.
                        # proj(qb+1) is emitted between attn(qb) and
                        # outproj(qb) so the PE queue has independent work
                        # while the softmax-normalize chain completes.  At the
                        # rep boundary the NEXT rep's proj(0) plays that role,
                        # so reps chain without a pipeline drain.
                        for rep in range(repeat):
                            if rep == 0:
                                emit_proj(0, load_w=True)
                            for qb in range(NQB):
                                emit_attn(qb, load_w=(rep == 0 and qb == 0))
                                if qb + 1 < NQB:
                                    emit_proj(qb + 1)
                                elif rep + 1 < repeat:
                                    emit_proj(0)
                                emit_outproj(qb)
                    else:
                        # attention needs the full K/V: project everything first
                        for rep in range(repeat):
                            for qb in range(NQB):
                                emit_proj(qb, load_w=(rep == 0 and qb == 0))
                            for qb in range(NQB):
                                emit_attn(qb, load_w=(rep == 0 and qb == 0))
                                emit_outproj(qb)
    nc.finalize()
    return nc


def _get_program(is_causal, with_bqk, with_bv, repeat=1):
    key = (bool(is_causal), bool(with_bqk), bool(with_bv), repeat)
    if key not in _prog_cache:
        _prog_cache[key] = _build(*key)
    return _prog_cache[key]


def _bf16(x):
    import ml_dtypes

    return np.ascontiguousarray(x).astype(ml_dtypes.bfloat16)


def _make_masks():
    i = np.arange(128)[:, None]
    j = np.arange(128)[None, :]
    tri = (j >= i).astype(np.float32)
    return np.concatenate([tri, tri], axis=1)  # [128, 2*128]


def _make_in_maps(Q_in, K_in, V_in, Wq, bq, Wk, bk, Wv, bv, Wo, bo, is_causal):
    with_bqk = bool(np.any(bq) or np.any(bk))
    with_bv = bool(np.any(bv))
    masks = _bf16(_make_masks()) if is_causal else None

    def shuf_qk(x):
        # X^T [(k p), (qb s)] -> [p, (qb k s)] so each stage DMA is contiguous
        return _bf16(
            x.T.reshape(KC, 128, NQB, QB).transpose(1, 2, 0, 3).reshape(128, -1)
        )

    def shuf_v(x):
        # X^T [(k p), (qt c)] -> [p, (qt k c)]
        return _bf16(
            x.T.reshape(KC, 128, NQT, 128).transpose(1, 2, 0, 3).reshape(128, -1)
        )

    def shuf_w(w):
        # W slice [dl, D] -> W^T [(k p), dl] -> [p, (k dl)]
        return _bf16(
            w.T.reshape(KC, 128, DL).transpose(1, 0, 2).reshape(128, -1)
        )

    xT = {}
    for b in range(BG):
        xT[("q", b)] = shuf_qk(Q_in[b])
        xT[("k", b)] = shuf_qk(K_in[b])
        xT[("v", b)] = shuf_v(V_in[b])
    in_maps = []
    for core in range(NCORES):
        b, hg = core // HG, core % HG
        sl = slice(DL * hg, DL * (hg + 1))
        m = {
            "xqT": xT[("q", b)],
            "xkT": xT[("k", b)],
            "xvT": xT[("v", b)],
            "wqT": shuf_w(Wq[sl, :]),
            "wkT": shuf_w(Wk[sl, :]),
            "wvT": shuf_w(Wv[sl, :]),
            "woT": _bf16(Wo[:, sl].T),
        }
        if is_causal:
            m["masks"] = masks
        if with_bqk:
            m["bqk"] = np.concatenate([bq[sl], bk[sl]]).astype(np.float32)[:, None]
        if with_bv:
            m["bvb"] = np.broadcast_to(bv[sl], (128, DL)).astype(np.float32).copy()
        in_maps.append(m)
    return in_maps, with_bqk, with_bv


def kernel(Q_in, K_in, V_in, Wq, bq, Wk, bk, Wv, bv, Wo, bo, is_causal):
    Q_in, K_in, V_in = (np.asarray(a, np.float32) for a in (Q_in, K_in, V_in))
    Wq, Wk, Wv, Wo = (np.asarray(a, np.float32) for a in (Wq, Wk, Wv, Wo))
    bq, bk, bv, bo = (np.asarray(a, np.float32) for a in (bq, bk, bv, bo))
    causal = bool(int(np.asarray(is_causal)))

    in_maps, with_bqk, with_bv = _make_in_maps(
        Q_in, K_in, V_in, Wq, bq, Wk, bk, Wv, bv, Wo, bo, causal
    )
    nc = _get_program(causal, with_bqk, with_bv)
    res = run_bass_kernel_spmd(nc, in_maps, list(range(NCORES)))
    out = np.zeros((B, S, D), np.float32)
    for core in range(NCORES):
        out[core // HG] += res.results[core]["y"]
    out += bo
    return out


# revision 15
# speedup vs baseline: 14.6075x; 1.0382x over previous
"""Trainium2 Bass kernel for nn_MultiHeadAttention (B=2, S=2048, D=1024, H=16).

Sharding: 8 cores = 2 batch groups x 4 head groups (4 heads / core).
Host pre-transposes X and the weight slices (and converts them to bf16) so the
device kernel needs no on-chip transposes:
  - activations flow feature-major (Q^T, K^T [256, 2048]) through scores,
  - scores are computed transposed (S^T[kk, q]) so softmax-sum comes from a
    ones-column augmented into V via the PV matmul itself,
  - head pairs share one 2-bank PSUM score tile so each exp activation covers
    both heads; causal boundary masks run on the otherwise-idle gpsimd queue,
  - emission order interleaves proj(qb+1) between attention(qb) and the
    output projection(qb) so the softmax-normalize latency hides under PE work,
  - the output projection flips back to seq-major [2048, 1024] fp32 partials,
  - host sums the 4 head-group partials per batch and adds the output bias.
All matmuls run in bf16 (full PE rate); accumulation stays fp32 in PSUM.
"""

import os
import sys

for _p in ("/opt/trn_rl_repo",):
    if os.path.isdir(_p) and _p not in sys.path:
        sys.path.append(_p)

import numpy as np

import concourse.bacc as bacc
import concourse.mybir as mybir
from concourse.bass_utils import run_bass_kernel_spmd
from concourse.tile import TileContext

F32 = mybir.dt.float32
BF16 = mybir.dt.bfloat16

B, S, D, H, HD = 2, 2048, 1024, 16, 64
NCORES = 8
BG = 2                 # batch groups
HG = NCORES // BG      # head groups (cores per batch)
HL = H // HG           # heads per core = 4
DL = HL * HD           # local head dims = 256
KC = D // 128          # contraction chunks for the projections = 8
QB = 512               # q block (matmul free dim)
NQB = S // QB          # 4
KT = 128               # key tile (scores partition dim)
NKT = S // KT          # 16
QT = 128               # out-proj q tile
NQT = S // QT          # 16
SCALE = 1.0 / np.sqrt(HD)

_prog_cache = {}

# timing-probe knob (test-only): "noxdma" skips x reloads on reps > 0 so the
# per-rep slope isolates compute from HBM traffic.  Never set when grading.
_PROBE = os.environ.get("BASS_PROBE", "")


def _build(is_causal, with_bqk, with_bv, repeat=1):
    nc = bacc.Bacc(None, target_bir_lowering=False, debug=False)

    xq_d = nc.declare_dram_parameter("xqT", [128, NQB * KC * QB], BF16, isOutput=False)
    xk_d = nc.declare_dram_parameter("xkT", [128, NQB * KC * QB], BF16, isOutput=False)
    xv_d = nc.declare_dram_parameter("xvT", [128, NQT * KC * 128], BF16, isOutput=False)
    wq_d = nc.declare_dram_parameter("wqT", [128, KC * DL], BF16, isOutput=False)
    wk_d = nc.declare_dram_parameter("wkT", [128, KC * DL], BF16, isOutput=False)
    wv_d = nc.declare_dram_parameter("wvT", [128, KC * DL], BF16, isOutput=False)
    wo_d = nc.declare_dram_parameter("woT", [DL, D], BF16, isOutput=False)
    y_d = nc.declare_dram_parameter("y", [S, D], F32, isOutput=True)
    mask_d = None
    if is_causal:
        # triangle mask duplicated for the two PE row groups: [128, 2, 128]
        mask_d = nc.declare_dram_parameter("masks", [128, 2 * 128], BF16, isOutput=False)
    bqk_d = None
    if with_bqk:
        # rows 0..DL-1 = bq, rows DL..2DL-1 = bk (per-partition bias columns)
        bqk_d = nc.declare_dram_parameter("bqk", [2 * DL, 1], F32, isOutput=False)
    bv_d = None
    if with_bv:
        bv_d = nc.declare_dram_parameter("bvb", [128, DL], F32, isOutput=False)

    with TileContext(nc) as tc:
        with (
            tc.tile_pool(name="const", bufs=1) as cp,
            tc.tile_pool(name="stream", bufs=3) as sp,
            tc.tile_pool(name="exps", bufs=6) as ep,
            tc.tile_pool(name="yout", bufs=3) as yp,
            tc.tile_pool(name="small", bufs=3) as smp,
            tc.tile_pool(name="ppsum", bufs=1, space="PSUM") as pp,
        ):
            # ---- static loads -------------------------------------------------
            wq_sb = cp.tile([128, KC, DL], BF16, tag="wq", name="wq")
            wk_sb = cp.tile([128, KC, DL], BF16, tag="wk", name="wk")
            wv_sb = cp.tile([128, KC, DL], BF16, tag="wv", name="wv")
            for half in range(2):
                cols = slice(half * KC * DL // 2, (half + 1) * KC * DL // 2)
                nc.sync.dma_start(
                    out=wk_sb[:, half * KC // 2 : (half + 1) * KC // 2, :],
                    in_=wk_d[:, cols],
                )
            mask_sb = None
            if is_causal:
                mask_sb = cp.tile([128, 2, 128], BF16, tag="mask", name="mask")
            wo_sb = [cp.tile([128, D], BF16, tag=f"wo{c}", name=f"wo{c}") for c in range(2)]
            bqk_sb = None
            if with_bqk:
                bqk_sb = cp.tile([128, 4], F32, tag="bqk", name="bqk")
                nc.sync.dma_start(
                    out=bqk_sb[:], in_=bqk_d[:].rearrange("(c p) o -> p (c o)", p=128)
                )
            bv_sb = None
            if with_bv:
                bv_sb = cp.tile([128, DL], F32, tag="bvb", name="bvb")
                nc.sync.dma_start(out=bv_sb[:], in_=bv_d[:])

            # persistent activation tensors (feature-major)
            qT = [cp.tile([128, S], BF16, tag=f"qT{i}", name=f"qT{i}") for i in range(2)]
            kT = [cp.tile([128, S], BF16, tag=f"kT{i}", name=f"kT{i}") for i in range(2)]
            # V augmented with a ones column per head: [seq-tile, head, 65]
            v_aug = cp.tile([128, NKT, HL, HD + 1], BF16, tag="vaug", name="vaug")
            oT = [cp.tile([128, S], BF16, tag=f"oT{i}", name=f"oT{i}") for i in range(2)]
            ones_bf = cp.tile([128, NKT * HL], BF16, tag="ones", name="ones")
            nc.vector.memset(ones_bf[:], 1.0)
            nc.vector.tensor_copy(
                v_aug[:, :, :, HD : HD + 1],
                ones_bf[:].rearrange("p (a b) -> p a b", a=NKT)[:, :, :, None],
            )

            if True:
                if True:

                    def proj_chunks(qb, load_w=False, skip_x=False):
                        # ---- projection for q block qb, as a DMA prologue
                        # plus a list of thunks each emitting a short burst of
                        # PE matmuls.  The thunks get interleaved between
                        # attention tiles so the PE queue always holds work
                        # that does not depend on the exp chain.
                        chunks = []
                        xcbs = {}
                        for pname, x_d in (("k", xk_d), ("q", xq_d)):
                            if load_w and pname == "q":
                                nc.sync.dma_start(
                                    out=wq_sb[:].rearrange("p k d -> p (k d)"),
                                    in_=wq_d[:],
                                )
                            xcb = sp.tile([128, KC, QB], BF16, tag="xqk", name="xqk", bufs=3)
                            blk = KC * QB
                            nsplit = 4 if load_w else 2
                            if not skip_x:
                                for part in range(nsplit):
                                    hk = KC // nsplit
                                    nc.sync.dma_start(
                                        out=xcb[:, hk * part : hk * (part + 1), :],
                                        in_=x_d[
                                            :,
                                            qb * blk + part * blk // nsplit : qb * blk
                                            + (part + 1) * blk // nsplit,
                                        ],
                                    )
                            xcbs[pname] = xcb
                        if load_w:
                            nc.sync.dma_start(
                                out=wv_sb[:].rearrange("p k d -> p (k d)"),
                                in_=wv_d[:],
                            )
                        xcols = []
                        for qi in range(4):
                            qt = 4 * qb + qi
                            xcol = sp.tile(
                                [128, KC, 128], BF16, tag="xcol", name="xcol", bufs=4
                            )
                            if not skip_x:
                                nc.sync.dma_start(
                                    out=xcol[:].rearrange("p k c -> p (k c)"),
                                    in_=xv_d[:, qt * KC * 128 : (qt + 1) * KC * 128],
                                )
                            xcols.append(xcol)

                        state = {}

                        def qk_chunk(pname, w_sb, out_tiles, bias_col, m, half):
                            xcb = xcbs[pname]
                            if half == 0:
                                state[(pname, m)] = pp.tile(
                                    [128, QB], F32, tag="pp", name="pp", bufs=2
                                )
                            pk = state[(pname, m)]
                            for kc in range(4 * half, 4 * half + 4):
                                nc.tensor.matmul(
                                    pk[:],
                                    w_sb[:, kc, 128 * m : 128 * (m + 1)],
                                    xcb[:, kc, :],
                                    start=(kc == 0),
                                    stop=(kc == KC - 1),
                                )
                            if half == 1:
                                dst = out_tiles[m][:, QB * qb : QB * (qb + 1)]
                                if with_bqk:
                                    nc.scalar.activation(
                                        dst,
                                        pk[:],
                                        mybir.ActivationFunctionType.Identity,
                                        bias=bqk_sb[
                                            :, 2 * bias_col + m : 2 * bias_col + m + 1
                                        ],
                                    )
                                else:
                                    nc.vector.tensor_copy(dst, pk[:])

                        def v_chunk(qi, half):
                            qt = 4 * qb + qi
                            if half == 0:
                                state[("v", qi)] = pp.tile(
                                    [128, DL], F32, tag="pp", name="pv", bufs=2
                                )
                            pv = state[("v", qi)]
                            for kc in range(4 * half, 4 * half + 4):
                                nc.tensor.matmul(
                                    pv[:],
                                    xcols[qi][:, kc, :],
                                    wv_sb[:, kc, :],
                                    start=(kc == 0),
                                    stop=(kc == KC - 1),
                                )
                            if half == 1:
                                vsrc = pv[:].rearrange("p (h d) -> p h d", h=HL)
                                vdst = v_aug[:, qt, :, 0:HD]
                                if with_bv:
                                    nc.vector.tensor_add(
                                        vdst,
                                        vsrc,
                                        bv_sb[:].rearrange("p (h d) -> p h d", h=HL),
                                    )
                                else:
                                    nc.vector.tensor_copy(vdst, vsrc)

                        from functools import partial

                        for m in range(2):
                            for half in range(2):
                                chunks.append(partial(qk_chunk, "k", wk_sb, kT, 1, m, half))
                        for m in range(2):
                            for half in range(2):
                                chunks.append(partial(qk_chunk, "q", wq_sb, qT, 0, m, half))
                        for qi in range(4):
                            for half in range(2):
                                chunks.append(partial(v_chunk, qi, half))
                        return chunks

                    def emit_attn(qb, chunks, load_w=False):
                        # ---- attention for this q block ----------------------
                        # Head pair (2*ht, 2*ht+1) computed concurrently on PE
                        # row groups (0,0)/(64,0) into one 2-bank PSUM tile so
                        # a single exp covers both.  The t-loop is software
                        # pipelined one stage deep (scores run a tile ahead of
                        # PV) and the next block's projection chunks fill the
                        # remaining PE slack so the exp chain never idles the
                        # PE (idle gaps trigger the HAM clock throttle on HW).
                        # Diagonal kk tiles narrow to the allowed q range; the
                        # boundary 128-col blockette gets the triangle mask
                        # (on gpsimd).
                        ntk = 4 * qb + 4 if is_causal else NKT
                        if load_w:
                            if is_causal:
                                nc.sync.dma_start(
                                    out=mask_sb[:].rearrange("p a b -> p (a b)"),
                                    in_=mask_d[:],
                                )
                            for c in range(2):
                                nc.sync.dma_start(
                                    out=wo_sb[c][:],
                                    in_=wo_d[128 * c : 128 * (c + 1), :],
                                )

                        nchunks = len(chunks)
                        total_slots = 2 * ntk
                        consumed = 0
                        slot = 0

                        def scores(ht, t):
                            qlo = max(0, 128 * (t - 4 * qb)) if is_causal else 0
                            ps = pp.tile([128, 2, QB], F32, tag="ps", name="ps", bufs=2)
                            for sub in range(2):
                                hr = 64 * sub
                                nc.tensor.matmul(
                                    ps[:, sub, qlo:QB],
                                    kT[ht][hr : hr + 64, 128 * t : 128 * (t + 1)],
                                    qT[ht][hr : hr + 64, QB * qb + qlo : QB * (qb + 1)],
                                    start=True,
                                    stop=True,
                                    tile_position=(hr, 0),
                                )
                            es = ep.tile([128, 2, QB], BF16, tag="es", name="es")
                            nc.scalar.activation(
                                es[:, :, qlo:QB],
                                ps[:, :, qlo:QB],
                                mybir.ActivationFunctionType.Exp,
                                scale=SCALE,
                            )
                            if is_causal and t >= 4 * qb:
                                nc.gpsimd.tensor_mul(
                                    es[:, :, qlo : qlo + 128],
                                    es[:, :, qlo : qlo + 128],
                                    mask_sb[:],
                                )
                            return es, qlo

                        for ht in range(2):
                            po_t = [
                                pp.tile(
                                    [HD + 1, QB], F32, tag="po", name="po", bufs=2
                                )
                                for _ in range(2)
                            ]
                            pend = scores(ht, 0)
                            for t in range(ntk):
                                es, qlo = pend
                                if t + 1 < ntk:
                                    pend = scores(ht, t + 1)
                                for sub in range(2):
                                    h = 2 * ht + sub
                                    nc.tensor.matmul(
                                        po_t[sub][:, qlo:QB],
                                        v_aug[:, t, h, :],
                                        es[:, sub, qlo:QB],
                                        start=(t == 0),
                                        stop=(t == ntk - 1),
                                    )
                                target = min(nchunks, (slot + 1) * nchunks // total_slots)
                                while consumed < target:
                                    chunks[consumed]()
                                    consumed += 1
                                slot += 1
                            # rows 0..63 are O^T, row 64 is the softmax sum
                            for sub in range(2):
                                hr = 64 * sub
                                r = smp.tile([1, QB], F32, tag="r", name="r")
                                nc.vector.reciprocal(r[:], po_t[sub][HD : HD + 1, :])
                                rb = smp.tile([64, QB], F32, tag="rb", name="rb")
                                nc.gpsimd.partition_broadcast(rb[:], r[0:1, :])
                                nc.vector.tensor_mul(
                                    oT[ht][hr : hr + 64, QB * qb : QB * (qb + 1)],
                                    po_t[sub][0:HD, :],
                                    rb[:],
                                )
                        while consumed < nchunks:
                            chunks[consumed]()
                            consumed += 1

                    def emit_outproj(qb):
                        # ---- output projection for this q block --------------
                        for qi in range(4):
                            qt = 4 * qb + qi
                            yt = yp.tile([128, D], F32, tag="yt", name="yt")
                            for nb2 in range(2):
                                py = pp.tile([128, QB], F32, tag="pp", name="py", bufs=2)
                                for c in range(2):
                                    nc.tensor.matmul(
                                        py[:],
                                        oT[c][:, 128 * qt : 128 * (qt + 1)],
                                        wo_sb[c][:, 512 * nb2 : 512 * (nb2 + 1)],
                                        start=(c == 0),
                                        stop=(c == 1),
                                    )
                                nc.vector.tensor_copy(
                                    yt[:, 512 * nb2 : 512 * (nb2 + 1)], py[:]
                                )
                            if _PROBE != "nodma":
                                nc.sync.dma_start(
                                    out=y_d[128 * qt : 128 * (qt + 1), :], in_=yt[:]
                                )

                    def emit_proj(qb, load_w=False):
                        for c in proj_chunks(qb, load_w=load_w):
                            c()

                    if is_causal:
                        # streaming: attention(qb) only needs K/V up to qb.
                        # proj(qb+1)'s matmul chunks are interleaved through
                        # attn(qb)'s tile loop; at the rep boundary the NEXT
                        # rep's proj(0) plays that role, so reps chain without
                        # a pipeline drain.
                        probe_skip = _PROBE in ("noxdma", "nodma")
                        for rep in range(repeat):
                            if rep == 0:
                                emit_proj(0, load_w=True)
                            for qb in range(NQB):
                                if qb + 1 < NQB:
                                    chunks = proj_chunks(
                                        qb + 1, skip_x=(probe_skip and rep > 0)
                                    )
                                elif rep + 1 < repeat:
                                    chunks = proj_chunks(0, skip_x=probe_skip)
                                else:
                                    chunks = []
                                emit_attn(qb, chunks, load_w=(rep == 0 and qb == 0))
                                emit_outproj(qb)
                    else:
                        # attention needs the full K/V: project everything first
                        for rep in range(repeat):
                            for qb in range(NQB):
                                emit_proj(qb, load_w=(rep == 0 and qb == 0))
                            for qb in range(NQB):
                                emit_attn(qb, [], load_w=(rep == 0 and qb == 0))
                                emit_outproj(qb)
    nc.finalize()
    return nc


def _get_program(is_causal, with_bqk, with_bv, repeat=1):
    key = (bool(is_causal), bool(with_bqk), bool(with_bv), repeat)
    if key not in _prog_cache:
        _prog_cache[key] = _build(*key)
    return _prog_cache[key]


def _bf16(x):
    import ml_dtypes

    return np.ascontiguousarray(x).astype(ml_dtypes.bfloat16)


def _make_masks():
    i = np.arange(128)[:, None]
    j = np.arange(128)[None, :]
    tri = (j >= i).astype(np.float32)
    return np.concatenate([tri, tri], axis=1)  # [128, 2*128]


def _make_in_maps(Q_in, K_in, V_in, Wq, bq, Wk, bk, Wv, bv, Wo, bo, is_causal):
    with_bqk = bool(np.any(bq) or np.any(bk))
    with_bv = bool(np.any(bv))
    masks = _bf16(_make_masks()) if is_causal else None

    def shuf_qk(x):
        # X^T [(k p), (qb s)] -> [p, (qb k s)] so each stage DMA is contiguous
        return _bf16(
            x.T.reshape(KC, 128, NQB, QB).transpose(1, 2, 0, 3).reshape(128, -1)
        )

    def shuf_v(x):
        # X^T [(k p), (qt c)] -> [p, (qt k c)]
        return _bf16(
            x.T.reshape(KC, 128, NQT, 128).transpose(1, 2, 0, 3).reshape(128, -1)
        )

    def shuf_w(w):
        # W slice [dl, D] -> W^T [(k p), dl] -> [p, (k dl)]
        return _bf16(
            w.T.reshape(KC, 128, DL).transpose(1, 0, 2).reshape(128, -1)
        )

    xT = {}
    for b in range(BG):
        xT[("q", b)] = shuf_qk(Q_in[b])
        xT[("k", b)] = shuf_qk(K_in[b])
        xT[("v", b)] = shuf_v(V_in[b])
    in_maps = []
    for core in range(NCORES):
        b, hg = core // HG, core % HG
        sl = slice(DL * hg, DL * (hg + 1))
        m = {
            "xqT": xT[("q", b)],
            "xkT": xT[("k", b)],
            "xvT": xT[("v", b)],
            "wqT": shuf_w(Wq[sl, :]),
            "wkT": shuf_w(Wk[sl, :]),
            "wvT": shuf_w(Wv[sl, :]),
            "woT": _bf16(Wo[:, sl].T),
        }
        if is_causal:
            m["masks"] = masks
        if with_bqk:
            m["bqk"] = np.concatenate([bq[sl], bk[sl]]).astype(np.float32)[:, None]
        if with_bv:
            m["bvb"] = np.broadcast_to(bv[sl], (128, DL)).astype(np.float32).copy()
        in_maps.append(m)
    return in_maps, with_bqk, with_bv


def kernel(Q_in, K_in, V_in, Wq, bq, Wk, bk, Wv, bv, Wo, bo, is_causal):
    Q_in, K_in, V_in = (np.asarray(a, np.float32) for a in (Q_in, K_in, V_in))
    Wq, Wk, Wv, Wo = (np.asarray(a, np.float32) for a in (Wq, Wk, Wv, Wo))
    bq, bk, bv, bo = (np.asarray(a, np.float32) for a in (bq, bk, bv, bo))
    causal = bool(int(np.asarray(is_causal)))

    in_maps, with_bqk, with_bv = _make_in_maps(
        Q_in, K_in, V_in, Wq, bq, Wk, bk, Wv, bv, Wo, bo, causal
    )
    nc = _get_program(causal, with_bqk, with_bv)
    res = run_bass_kernel_spmd(nc, in_maps, list(range(NCORES)))
    out = np.zeros((B, S, D), np.float32)
    for core in range(NCORES):
        out[core // HG] += res.results[core]["y"]
    out += bo
    return out


# revision 29
# speedup vs baseline: 17.5780x; 1.2034x over previous
"""Trainium2 Bass kernel for nn_MultiHeadAttention (B=2, S=2048, D=1024, H=16).

Sharding: 8 cores = 2 batch groups x 4 head groups (4 heads / core).
Host pre-transposes X and the weight slices (and converts them to bf16) so the
device kernel needs no on-chip transposes:
  - activations flow feature-major (Q^T, K^T [256, 2048]) through scores,
  - scores are computed transposed (S^T[kk, q]) so softmax-sum comes from a
    ones-column augmented into V via the PV matmul itself,
  - head pairs share one 2-bank PSUM score tile so each exp activation covers
    both heads; causal boundary masks run on the otherwise-idle gpsimd queue,
  - emission order interleaves proj(qb+1) between attention(qb) and the
    output projection(qb) so the softmax-normalize latency hides under PE work,
  - the output projection flips back to seq-major [2048, 1024] fp32 partials,
  - host sums the 4 head-group partials per batch and adds the output bias.
All matmuls run in bf16 (full PE rate); accumulation stays fp32 in PSUM.
"""

import os
import sys

for _p in ("/opt/trn_rl_repo",):
    if os.path.isdir(_p) and _p not in sys.path:
        sys.path.append(_p)

import numpy as np

import concourse.bacc as bacc
import concourse.mybir as mybir
from concourse.bass_utils import run_bass_kernel_spmd
from concourse.tile import TileContext

F32 = mybir.dt.float32
BF16 = mybir.dt.bfloat16

B, S, D, H, HD = 2, 2048, 1024, 16, 64
NCORES = 8
BG = 2                 # batch groups
HG = NCORES // BG      # head groups (cores per batch)
HL = H // HG           # heads per core = 4
DL = HL * HD           # local head dims = 256
KC = D // 128          # contraction chunks for the projections = 8
QB = 512               # q block (matmul free dim)
NQB = S // QB          # 4
KT = 128               # key tile (scores partition dim)
NKT = S // KT          # 16
QT = 128               # out-proj q tile
NQT = S // QT          # 16
SCALE = 1.0 / np.sqrt(HD)

_prog_cache = {}

# timing-probe knob (test-only): "noxdma" skips x reloads on reps > 0 so the
# per-rep slope isolates compute from HBM traffic.  Never set when grading.
_PROBE = os.environ.get("BASS_PROBE", "")


def _build(is_causal, with_bqk, with_bv, repeat=1):
    nc = bacc.Bacc(None, target_bir_lowering=False, debug=False)

    xq_d = nc.declare_dram_parameter("xqT", [128, NQB * KC * QB], BF16, isOutput=False)
    xk_d = nc.declare_dram_parameter("xkT", [128, NQB * KC * QB], BF16, isOutput=False)
    xv_d = nc.declare_dram_parameter("xvT", [128, NQT * KC * 128], BF16, isOutput=False)
    wq_d = nc.declare_dram_parameter("wqT", [128, KC * DL], BF16, isOutput=False)
    wk_d = nc.declare_dram_parameter("wkT", [128, KC * DL], BF16, isOutput=False)
    wv_d = nc.declare_dram_parameter("wvT", [128, KC * DL], BF16, isOutput=False)
    wo_d = nc.declare_dram_parameter("woT", [DL, D], BF16, isOutput=False)
    # bf16 partials: host sums the 4 head-group partials in fp32; the extra
    # rounding is ~0.2% of each partial, well inside the error budget, and it
    # halves the output DMA traffic
    y_d = nc.declare_dram_parameter("y", [S, D], BF16, isOutput=True)
    mask_d = None
    if is_causal:
        # triangle mask duplicated for the two PE row groups: [128, 2, 128]
        mask_d = nc.declare_dram_parameter("masks", [128, 2 * 128], BF16, isOutput=False)
    bqk_d = None
    if with_bqk:
        # rows 0..DL-1 = bq, rows DL..2DL-1 = bk (per-partition bias columns)
        bqk_d = nc.declare_dram_parameter("bqk", [2 * DL, 1], F32, isOutput=False)
    bv_d = None
    if with_bv:
        bv_d = nc.declare_dram_parameter("bvb", [128, DL], F32, isOutput=False)

    with TileContext(nc) as tc:
        with (
            tc.tile_pool(name="const", bufs=1) as cp,
            tc.tile_pool(name="stream", bufs=3) as sp,
            tc.tile_pool(name="exps", bufs=6) as ep,
            tc.tile_pool(name="yout", bufs=3) as yp,
            tc.tile_pool(name="small", bufs=3) as smp,
            tc.tile_pool(name="ppsum", bufs=1, space="PSUM") as pp,
        ):
            # ---- static loads -------------------------------------------------
            wq_sb = cp.tile([128, KC, DL], BF16, tag="wq", name="wq")
            wk_sb = cp.tile([128, KC, DL], BF16, tag="wk", name="wk")
            wv_sb = cp.tile([128, KC, DL], BF16, tag="wv", name="wv")
            for half in range(2):
                cols = slice(half * KC * DL // 2, (half + 1) * KC * DL // 2)
                nc.sync.dma_start(
                    out=wk_sb[:, half * KC // 2 : (half + 1) * KC // 2, :],
                    in_=wk_d[:, cols],
                )
            mask_sb = None
            if is_causal:
                mask_sb = cp.tile([128, 2, 128], BF16, tag="mask", name="mask")
            wo_sb = [cp.tile([128, D], BF16, tag=f"wo{c}", name=f"wo{c}") for c in range(2)]
            bqk_sb = None
            if with_bqk:
                bqk_sb = cp.tile([128, 4], F32, tag="bqk", name="bqk")
                nc.sync.dma_start(
                    out=bqk_sb[:], in_=bqk_d[:].rearrange("(c p) o -> p (c o)", p=128)
                )
            bv_sb = None
            if with_bv:
                bv_sb = cp.tile([128, DL], F32, tag="bvb", name="bvb")
                nc.sync.dma_start(out=bv_sb[:], in_=bv_d[:])

            # persistent activation tensors (feature-major)
            qT = [cp.tile([128, S], BF16, tag=f"qT{i}", name=f"qT{i}") for i in range(2)]
            kT = [cp.tile([128, S], BF16, tag=f"kT{i}", name=f"kT{i}") for i in range(2)]
            # V augmented with a ones column per head: [seq-tile, head, 65]
            v_aug = cp.tile([128, NKT, HL, HD + 1], BF16, tag="vaug", name="vaug")
            oT = [cp.tile([128, S], BF16, tag=f"oT{i}", name=f"oT{i}") for i in range(2)]
            es_const = None
            if _PROBE == "nochain":
                es_const = cp.tile([128, 2, QB], BF16, tag="esc", name="esc")
                nc.vector.memset(es_const[:].rearrange("p a b -> p (a b)"), 0.001)
            xcb_const = xcol_const = None
            if _PROBE in ("noxdma", "nodma"):
                xcb_const = cp.tile([128, KC, QB], BF16, tag="xcbc", name="xcbc")
                nc.vector.memset(xcb_const[:].rearrange("p a b -> p (a b)"), 0.001)
                xcol_const = cp.tile([128, KC, 128], BF16, tag="xclc", name="xclc")
                nc.vector.memset(xcol_const[:].rearrange("p a b -> p (a b)"), 0.001)
            ones_bf = cp.tile([128, NKT * HL], BF16, tag="ones", name="ones")
            nc.vector.memset(ones_bf[:], 1.0)
            nc.vector.tensor_copy(
                v_aug[:, :, :, HD : HD + 1],
                ones_bf[:].rearrange("p (a b) -> p a b", a=NKT)[:, :, :, None],
            )

            if True:
                if True:

                    def proj_chunks(qb, load_w=False, skip_x=False):
                        # ---- projection for q block qb, as a DMA prologue
                        # plus a list of thunks each emitting a short burst of
                        # PE matmuls.  The thunks get interleaved between
                        # attention tiles so the PE queue always holds work
                        # that does not depend on the exp chain.
                        chunks = []
                        xcbs = {}
                        for pname, x_d in (("k", xk_d), ("q", xq_d)):
                            if load_w and pname == "q":
                                nc.sync.dma_start(
                                    out=wq_sb[:].rearrange("p k d -> p (k d)"),
                                    in_=wq_d[:],
                                )
                            if skip_x:
                                xcbs[pname] = xcb_const
                                continue
                            xcb = sp.tile([128, KC, QB], BF16, tag="xqk", name="xqk", bufs=3)
                            blk = KC * QB
                            nsplit = 4 if load_w else 2
                            for part in range(nsplit):
                                hk = KC // nsplit
                                nc.sync.dma_start(
                                    out=xcb[:, hk * part : hk * (part + 1), :],
                                    in_=x_d[
                                        :,
                                        qb * blk + part * blk // nsplit : qb * blk
                                        + (part + 1) * blk // nsplit,
                                    ],
                                )
                            xcbs[pname] = xcb
                        if load_w:
                            nc.sync.dma_start(
                                out=wv_sb[:].rearrange("p k d -> p (k d)"),
                                in_=wv_d[:],
                            )
                        xcols = []
                        for qi in range(4):
                            if skip_x:
                                xcols.append(xcol_const)
                                continue
                            qt = 4 * qb + qi
                            xcol = sp.tile(
                                [128, KC, 128], BF16, tag="xcol", name="xcol", bufs=4
                            )
                            nc.sync.dma_start(
                                out=xcol[:].rearrange("p k c -> p (k c)"),
                                in_=xv_d[:, qt * KC * 128 : (qt + 1) * KC * 128],
                            )
                            xcols.append(xcol)

                        state = {}

                        def ppab():
                            # two single-bank slots; chains alternate between
                            # them MM-by-MM so successive matmuls never hit
                            # the same PSUM bank (same-bank write-after-write
                            # stalls the PE pipe)
                            return (
                                pp.tile([128, QB], F32, tag="ppA", name="ppA", bufs=1),
                                pp.tile([128, QB], F32, tag="ppB", name="ppB", bufs=1),
                            )

                        def qk_chunk(pname, w_sb, out_tiles, bias_col, half):
                            xcb = xcbs[pname]
                            if half == 0:
                                state[pname] = ppab()
                            pks = state[pname]
                            for kc in range(4 * half, 4 * half + 4):
                                for m in range(2):
                                    nc.tensor.matmul(
                                        pks[m][:],
                                        w_sb[:, kc, 128 * m : 128 * (m + 1)],
                                        xcb[:, kc, :],
                                        start=(kc == 0),
                                        stop=(kc == KC - 1),
                                    )
                            if half == 1:
                                for m in range(2):
                                    dst = out_tiles[m][:, QB * qb : QB * (qb + 1)]
                                    if with_bqk:
                                        nc.scalar.activation(
                                            dst,
                                            pks[m][:],
                                            mybir.ActivationFunctionType.Identity,
                                            bias=bqk_sb[
                                                :, 2 * bias_col + m : 2 * bias_col + m + 1
                                            ],
                                        )
                                    else:
                                        nc.vector.tensor_copy(dst, pks[m][:])

                        def v_chunk(qp, half):
                            # two v-tiles interleaved across the two banks
                            if half == 0:
                                state[("v", qp)] = ppab()
                            pvs = state[("v", qp)]
                            for kc in range(4 * half, 4 * half + 4):
                                for j in range(2):
                                    nc.tensor.matmul(
                                        pvs[j][:, 0:DL],
                                        xcols[2 * qp + j][:, kc, :],
                                        wv_sb[:, kc, :],
                                        start=(kc == 0),
                                        stop=(kc == KC - 1),
                                    )
                            if half == 1:
                                for j in range(2):
                                    qt = 4 * qb + 2 * qp + j
                                    vsrc = pvs[j][:, 0:DL].rearrange(
                                        "p (h d) -> p h d", h=HL
                                    )
                                    vdst = v_aug[:, qt, :, 0:HD]
                                    if with_bv:
                                        nc.vector.tensor_add(
                                            vdst,
                                            vsrc,
                                            bv_sb[:].rearrange("p (h d) -> p h d", h=HL),
                                        )
                                    else:
                                        nc.vector.tensor_copy(vdst, vsrc)

                        from functools import partial

                        for half in range(2):
                            chunks.append(partial(qk_chunk, "k", wk_sb, kT, 1, half))
                        for half in range(2):
                            chunks.append(partial(qk_chunk, "q", wq_sb, qT, 0, half))
                        for qp in range(2):
                            for half in range(2):
                                chunks.append(partial(v_chunk, qp, half))
                        return chunks

                    def emit_attn(qb, chunks, load_w=False):
                        # ---- attention for this q block ----------------------
                        # Head pair (2*ht, 2*ht+1) computed concurrently on PE
                        # row groups (0,0)/(64,0) into one 2-bank PSUM tile so
                        # a single exp covers both.  The t-loop is software
                        # pipelined one stage deep (scores run a tile ahead of
                        # PV) and the next block's projection chunks fill the
                        # remaining PE slack so the exp chain never idles the
                        # PE (idle gaps trigger the HAM clock throttle on HW).
                        # Diagonal kk tiles narrow to the allowed q range; the
                        # boundary 128-col blockette gets the triangle mask
                        # (on gpsimd).
                        ntk = 4 * qb + 4 if is_causal else NKT
                        if load_w:
                            if is_causal:
                                nc.sync.dma_start(
                                    out=mask_sb[:].rearrange("p a b -> p (a b)"),
                                    in_=mask_d[:],
                                )
                            for c in range(2):
                                nc.sync.dma_start(
                                    out=wo_sb[c][:],
                                    in_=wo_d[128 * c : 128 * (c + 1), :],
                                )

                        nchunks = len(chunks)
                        total_slots = 2 * ntk
                        consumed = 0
                        slot = 0

                        def scores(ht, t):
                            qlo = max(0, 128 * (t - 4 * qb)) if is_causal else 0
                            ps = pp.tile([128, 2, QB], F32, tag="ps", name="ps", bufs=2)
                            for sub in range(2):
                                hr = 64 * sub
                                nc.tensor.matmul(
                                    ps[:, sub, qlo:QB],
                                    kT[ht][hr : hr + 64, 128 * t : 128 * (t + 1)],
                                    qT[ht][hr : hr + 64, QB * qb + qlo : QB * (qb + 1)],
                                    start=True,
                                    stop=True,
                                    tile_position=(hr, 0),
                                )
                            es = ep.tile([128, 2, QB], BF16, tag="es", name="es")
                            nc.scalar.activation(
                                es[:, :, qlo:QB],
                                ps[:, :, qlo:QB],
                                mybir.ActivationFunctionType.Exp,
                                scale=SCALE,
                            )
                            if is_causal and t >= 4 * qb:
                                eng = nc.vector if t % 2 == 0 else nc.gpsimd
                                eng.tensor_mul(
                                    es[:, :, qlo : qlo + 128],
                                    es[:, :, qlo : qlo + 128],
                                    mask_sb[:],
                                )
                            if es_const is not None:
                                return es_const, qlo
                            return es, qlo

                        for ht in range(2):
                            po_t = [
                                pp.tile(
                                    [HD + 1, QB], F32, tag="po", name="po", bufs=2
                                )
                                for _ in range(2)
                            ]
                            pend = scores(ht, 0)
                            for t in range(ntk):
                                es, qlo = pend
                                if t + 1 < ntk:
                                    pend = scores(ht, t + 1)
                                for sub in range(2):
                                    h = 2 * ht + sub
                                    nc.tensor.matmul(
                                        po_t[sub][:, qlo:QB],
                                        v_aug[:, t, h, :],
                                        es[:, sub, qlo:QB],
                                        start=(t == 0),
                                        stop=(t == ntk - 1),
                                    )
                                target = min(nchunks, (slot + 1) * nchunks // total_slots)
                                while consumed < target:
                                    chunks[consumed]()
                                    consumed += 1
                                slot += 1
                            # rows 0..63 are O^T, row 64 is the softmax sum
                            for sub in range(2):
                                hr = 64 * sub
                                r = smp.tile([1, QB], F32, tag="r", name="r")
                                nc.vector.reciprocal(r[:], po_t[sub][HD : HD + 1, :])
                                rb = smp.tile([64, QB], F32, tag="rb", name="rb")
                                nc.gpsimd.partition_broadcast(rb[:], r[0:1, :])
                                nc.vector.tensor_mul(
                                    oT[ht][hr : hr + 64, QB * qb : QB * (qb + 1)],
                                    po_t[sub][0:HD, :],
                                    rb[:],
                                )
                        while consumed < nchunks:
                            chunks[consumed]()
                            consumed += 1

                    def emit_outproj(qb):
                        # ---- output projection for this q block --------------
                        # nb2 halves interleaved across the two PSUM banks;
                        # copies alternate DVE/gpsimd to split the load
                        for qi in range(4):
                            qt = 4 * qb + qi
                            yt = yp.tile([128, D], BF16, tag="yt", name="yt")
                            pys = (
                                pp.tile([128, QB], F32, tag="ppA", name="pyA", bufs=1),
                                pp.tile([128, QB], F32, tag="ppB", name="pyB", bufs=1),
                            )
                            for c in range(2):
                                for nb2 in range(2):
                                    nc.tensor.matmul(
                                        pys[nb2][:],
                                        oT[c][:, 128 * qt : 128 * (qt + 1)],
                                        wo_sb[c][:, 512 * nb2 : 512 * (nb2 + 1)],
                                        start=(c == 0),
                                        stop=(c == 1),
                                    )
                            for nb2 in range(2):
                                nc.vector.tensor_copy(
                                    yt[:, 512 * nb2 : 512 * (nb2 + 1)], pys[nb2][:]
                                )
                            if _PROBE != "nodma":
                                nc.sync.dma_start(
                                    out=y_d[128 * qt : 128 * (qt + 1), :], in_=yt[:]
                                )

                    def emit_proj(qb, load_w=False):
                        for c in proj_chunks(qb, load_w=load_w):
                            c()

                    if is_causal:
                        # streaming: attention(qb) only needs K/V up to qb.
                        # proj(qb+1)'s matmul chunks are interleaved through
                        # attn(qb)'s tile loop; at the rep boundary the NEXT
                        # rep's proj(0) plays that role, so reps chain without
                        # a pipeline drain.
                        probe_skip = _PROBE in ("noxdma", "nodma")
                        for rep in range(repeat):
                            if rep == 0:
                                emit_proj(0, load_w=True)
                            for qb in range(NQB):
                                if qb + 1 < NQB:
                                    chunks = proj_chunks(
                                        qb + 1, skip_x=(probe_skip and rep > 0)
                                    )
                                elif rep + 1 < repeat:
                                    chunks = proj_chunks(0, skip_x=probe_skip)
                                else:
                                    chunks = []
                                emit_attn(qb, chunks, load_w=(rep == 0 and qb == 0))
                                emit_outproj(qb)
                    else:
                        # attention needs the full K/V: project everything first
                        for rep in range(repeat):
                            for qb in range(NQB):
                                emit_proj(qb, load_w=(rep == 0 and qb == 0))
                            for qb in range(NQB):
                                emit_attn(qb, [], load_w=(rep == 0 and qb == 0))
                                emit_outproj(qb)
    nc.finalize()
    return nc


def _get_program(is_causal, with_bqk, with_bv, repeat=1):
    key = (bool(is_causal), bool(with_bqk), bool(with_bv), repeat)
    if key not in _prog_cache:
        _prog_cache[key] = _build(*key)
    return _prog_cache[key]


def _bf16(x):
    import ml_dtypes

    return np.ascontiguousarray(x).astype(ml_dtypes.bfloat16)


def _make_masks():
    i = np.arange(128)[:, None]
    j = np.arange(128)[None, :]
    tri = (j >= i).astype(np.float32)
    return np.concatenate([tri, tri], axis=1)  # [128, 2*128]


def _make_in_maps(Q_in, K_in, V_in, Wq, bq, Wk, bk, Wv, bv, Wo, bo, is_causal):
    with_bqk = bool(np.any(bq) or np.any(bk))
    with_bv = bool(np.any(bv))
    masks = _bf16(_make_masks()) if is_causal else None

    def shuf_qk(x):
        # X^T [(k p), (qb s)] -> [p, (qb k s)] so each stage DMA is contiguous
        return _bf16(
            x.T.reshape(KC, 128, NQB, QB).transpose(1, 2, 0, 3).reshape(128, -1)
        )

    def shuf_v(x):
        # X^T [(k p), (qt c)] -> [p, (qt k c)]
        return _bf16(
            x.T.reshape(KC, 128, NQT, 128).transpose(1, 2, 0, 3).reshape(128, -1)
        )

    def shuf_w(w):
        # W slice [dl, D] -> W^T [(k p), dl] -> [p, (k dl)]
        return _bf16(
            w.T.reshape(KC, 128, DL).transpose(1, 0, 2).reshape(128, -1)
        )

    xT = {}
    for b in range(BG):
        xT[("q", b)] = shuf_qk(Q_in[b])
        xT[("k", b)] = shuf_qk(K_in[b])
        xT[("v", b)] = shuf_v(V_in[b])
    in_maps = []
    for core in range(NCORES):
        b, hg = core // HG, core % HG
        sl = slice(DL * hg, DL * (hg + 1))
        m = {
            "xqT": xT[("q", b)],
            "xkT": xT[("k", b)],
            "xvT": xT[("v", b)],
            "wqT": shuf_w(Wq[sl, :]),
            "wkT": shuf_w(Wk[sl, :]),
            "wvT": shuf_w(Wv[sl, :]),
            "woT": _bf16(Wo[:, sl].T),
        }
        if is_causal:
            m["masks"] = masks
        if with_bqk:
            m["bqk"] = np.concatenate([bq[sl], bk[sl]]).astype(np.float32)[:, None]
        if with_bv:
            m["bvb"] = np.broadcast_to(bv[sl], (128, DL)).astype(np.float32).copy()
        in_maps.append(m)
    return in_maps, with_bqk, with_bv


def kernel(Q_in, K_in, V_in, Wq, bq, Wk, bk, Wv, bv, Wo, bo, is_causal):
    Q_in, K_in, V_in = (np.asarray(a, np.float32) for a in (Q_in, K_in, V_in))
    Wq, Wk, Wv, Wo = (np.asarray(a, np.float32) for a in (Wq, Wk, Wv, Wo))
    bq, bk, bv, bo = (np.asarray(a, np.float32) for a in (bq, bk, bv, bo))
    causal = bool(int(np.asarray(is_causal)))

    in_maps, with_bqk, with_bv = _make_in_maps(
        Q_in, K_in, V_in, Wq, bq, Wk, bk, Wv, bv, Wo, bo, causal
    )
    nc = _get_program(causal, with_bqk, with_bv)
    res = run_bass_kernel_spmd(nc, in_maps, list(range(NCORES)))
    out = np.zeros((B, S, D), np.float32)
    for core in range(NCORES):
        out[core // HG] += res.results[core]["y"].astype(np.float32)
    out += bo
    return out
